# revision 26
# baseline (speedup 1.0000x reference)
"""Two-layer GAT (DGL GATConv-style) on 8 Trainium2 NeuronCores via Bass/Tile.

Strategy (v4)
-------------
The per-execution cost through this runner is dominated by host<->device
I/O (inputs are re-shipped every execution; both byte count and tensor
count matter), so the kernel ships ONE ~0.9MiB int32 blob per core
(u16 gather row-ids, u8 dst-local offsets, bf16 features/weights, all
bitcast-sliced on device) and derives everything else on device:

* Node phase is SHARDED: core c projects only its own N/8 nodes
  (one matmul + one PSUM->SBUF copy + one contiguous 67KB write per
  128-node tile) into a local table shard tabs[w*128+p] =
  [h (256) | el (4) | er (4)], all bf16 (528B rows).  An AllGather
  replicates the 3.3MB shard so every core holds the full table; edge
  gathers then index row (c*WN*128 + w*128 + p) for node c*NPC+w*128+p
  (the SAME row mapping serves both layers, so one soff/dwin set).
* Edge phase: edges sorted by dst, windowed by 128 dst nodes, tiled by
  128 edges, chunked by G=32 tiles.  Per chunk: 32 row gathers (indirect
  DMA by src row), a [1, G*128] dloc row partition-broadcast-DMA'd and
  compared against the partition index to build the TRANSPOSED one-hot
  ohT (d-major), per-tile matmuls erp = ohT_t @ er_window broadcast
  er[dst] to edges, then chunk-wide DVE ops: scores, leaky-relu
  (scalar_tensor_tensor), one Exp, m = [h*ee | ee] via stride-0
  broadcast, forward one-hot oh, and per-tile accumulation
  psum_w += oh_t^T @ m_t.  Epilogue per window divides by the summed ee
  (exp(e)/sum exp(e) == softmax exactly), adds bias, tanh+head-mean (L1,
  written transposed for the layer-2 projection) or head-mean (L2).
* Layer 2 projects each core's OWN h1 windows directly (no activation
  collective) and AllGathers the second table shard.
"""

import sys
from contextlib import ExitStack

import numpy as np

sys.path.insert(0, "/opt/trn_rl_repo")

import concourse.bass as bass  # noqa: E402
import concourse.mybir as mybir  # noqa: E402
from concourse.bass import IndirectOffsetOnAxis  # noqa: E402
from concourse.bass_utils import run_bass_kernel_spmd  # noqa: E402
from concourse.masks import make_identity  # noqa: E402
from concourse.tile import TileContext  # noqa: E402

BF16 = mybir.dt.bfloat16
F32 = mybir.dt.float32
I32 = mybir.dt.int32
U16 = mybir.dt.uint16
U8 = mybir.dt.uint8
FP8 = mybir.dt.float8e4
NP_BF16 = mybir.dt.np(BF16)
NP_FP8 = mybir.dt.np(FP8)

AF = mybir.ActivationFunctionType
ALU = mybir.AluOpType

M_CORES = 8
NEG_SLOPE = 0.2
G_TILES = 32  # tiles (of 128 edges) per chunk


# ----------------------------------------------------------------------------
# Host-side preprocessing
# ----------------------------------------------------------------------------
class Cfg:
    pass


def _ceil_div(a, b):
    return -(-a // b)


def _prepare(x, src, dst, W1, al1, ar1, b1, W2, al2, ar2, b2, m_cores=M_CORES):
    cfg = Cfg()
    N, F = x.shape
    E = src.shape[0]
    H = al1.shape[0]
    assert N % m_cores == 0
    npc = N // m_cores
    wn = _ceil_div(npc, 128)
    wnp = wn * 128
    HF = H * F

    cfg.N, cfg.F, cfg.E, cfg.H, cfg.M = N, F, E, H, m_cores
    cfg.NPC, cfg.WN, cfg.WNP, cfg.HF = npc, wn, wnp, HF
    cfg.ROWC = HF + 2 * H  # bf16 cols: h | el | er
    cfg.MC = HF + H  # matmul rhs cols: scaled h | ee

    # unified table row id for node n (both layers)
    def rid(n):
        c = n // npc
        l = n % npc
        return c * wnp + (l // 128) * 128 + l % 128

    # ---- edge partition: sort by dst, split by dst range, window by 128 ----
    order = np.argsort(dst, kind="stable")
    ss = src[order].astype(np.int64)
    ds = dst[order].astype(np.int64)
    core = ds // npc
    dl = ds % npc
    win = dl // 128
    dloc = (dl - win * 128).astype(np.float32)

    grp = core * wn + win  # non-decreasing
    counts = np.bincount(grp, minlength=m_cores * wn).reshape(m_cores, wn)
    tw = np.maximum(1, _ceil_div(counts.max(axis=0), 128))  # tiles per window
    ttot = int(tw.sum())
    base = np.zeros(wn + 1, np.int64)
    base[1:] = np.cumsum(tw * 128)
    starts = np.searchsorted(grp, np.arange(m_cores * wn))
    ends = np.searchsorted(grp, np.arange(m_cores * wn) + 1)

    soff = np.zeros((m_cores, 128, ttot), np.uint16)
    dlocs = np.zeros((m_cores, 128, ttot), np.uint8)
    dlocf = np.zeros((m_cores, 1, ttot * 128), np.uint8)
    for c in range(m_cores):
        s_src = np.zeros(ttot * 128, np.int64)
        s_dlc = np.full(ttot * 128, 255, np.int64)
        for w in range(wn):
            s0, e0 = starts[c * wn + w], ends[c * wn + w]
            n = e0 - s0
            b0 = base[w]
            s_src[b0:b0 + n] = ss[s0:e0]
            s_dlc[b0:b0 + n] = dloc[s0:e0].astype(np.int64)
        soff[c] = rid(s_src).reshape(ttot, 128).T.astype(np.uint16)
        dlocs[c] = s_dlc.reshape(ttot, 128).T.astype(np.uint8)
        dlocf[c, 0] = s_dlc.astype(np.uint8)

    p_ar = np.arange(128)
    dwin = np.zeros((m_cores, 128, wn), np.int32)
    for c in range(m_cores):
        for w in range(wn):
            dw = min(128, npc - w * 128)
            dwin[c, :, w] = c * wnp + w * 128 + np.minimum(p_ar, dw - 1)

    cfg.TW = [int(t) for t in tw]
    cfg.TTOT = ttot
    win_of, first_t, last_t = [], [], []
    for w in range(wn):
        for i in range(cfg.TW[w]):
            win_of.append(w)
            first_t.append(i == 0)
            last_t.append(i == cfg.TW[w] - 1)
    cfg.win_of, cfg.first_t, cfg.last_t = win_of, first_t, last_t

    # ---- folded weights: [W | W.al | W.ar] -> [F, ROWC] bf16 ----
    def aug(Wm, al, ar):
        W64 = Wm.astype(np.float64).reshape(F, H, F)
        wal = np.einsum("khf,hf->kh", W64, al.astype(np.float64))
        war = np.einsum("khf,hf->kh", W64, ar.astype(np.float64))
        return np.concatenate(
            [Wm.astype(np.float64), wal, war], axis=1
        ).astype(NP_BF16)

    W1a = aug(W1, al1, ar1)
    W2a = aug(W2, al2, ar2)

    b1row = np.asarray(b1, np.float32).reshape(HF)
    b2row = np.asarray(b2, np.float32).reshape(HF)
    iotar = np.arange(128, dtype=np.float32)
    iotaP = np.arange(128, dtype=np.float32)

    # ---- pack everything into ONE int32 blob per core (per-exec transfer
    # cost scales with tensor COUNT as well as bytes) ----
    off = {}
    cur = 0

    def _add(name, nbytes):
        nonlocal cur
        cur = (cur + 511) // 512 * 512
        off[name] = cur
        cur += nbytes

    _add("soff", 128 * ttot * 2)
    _add("dloc", 128 * ttot * 1)
    _add("dlocf", ttot * 128 * 1)
    _add("xTt", wnp * F * 2)
    _add("W1", F * cfg.ROWC * 2)
    _add("W2", F * cfg.ROWC * 2)
    _add("b1", HF * 4)
    _add("b2", HF * 4)
    _add("iotar", 128 * 4)
    _add("iotaP", 128 * 4)
    _add("dwin", 128 * wn * 4)
    totb = (cur + 511) // 512 * 512
    cfg.off, cfg.TOTB = off, totb

    in_maps = []
    for c in range(m_cores):
        xs = np.zeros((wnp, F), np.float32)
        xs[:npc] = np.asarray(x[c * npc:(c + 1) * npc], np.float32)
        xTt = np.ascontiguousarray(
            xs.reshape(wn, 128, F).transpose(0, 2, 1)).astype(NP_BF16)
        buf = np.zeros(totb, np.uint8)

        def _put(name, arr):
            b = arr.tobytes()
            buf[off[name]:off[name] + len(b)] = np.frombuffer(b, np.uint8)

        _put("soff", np.ascontiguousarray(soff[c]))
        _put("dloc", np.ascontiguousarray(dlocs[c]))
        _put("dlocf", np.ascontiguousarray(dlocf[c]))
        _put("xTt", xTt)
        _put("W1", W1a)
        _put("W2", W2a)
        _put("b1", b1row)
        _put("b2", b2row)
        _put("iotar", iotar)
        _put("iotaP", iotaP)
        _put("dwin", np.ascontiguousarray(dwin[c]))
        in_maps.append(dict(blob=buf.view(np.int32).reshape(1, -1)))
    return cfg, in_maps


# ----------------------------------------------------------------------------
# Bass program
# ----------------------------------------------------------------------------
def build_program(cfg):
    N, F, H, M = cfg.N, cfg.F, cfg.H, cfg.M
    HF, NPC, WN, WNP = cfg.HF, cfg.NPC, cfg.WN, cfg.WNP
    ROWC, MC = cfg.ROWC, cfg.MC
    G = G_TILES
    TTOT = cfg.TTOT

    nc = bass.Bass(num_devices=M, enable_partition_id=False)

    O = cfg.off
    blob_d = nc.dram_tensor(
        "blob", [1, cfg.TOTB // 4], I32, kind="ExternalInput")
    bb = blob_d.bitcast(BF16)
    bi = blob_d
    bf = blob_d.bitcast(F32)
    bu16 = blob_d.bitcast(U16)
    bu8 = blob_d.bitcast(U8)
    bf8 = blob_d.bitcast(FP8)
    soff_v = bu16[0:1, O["soff"] // 2:O["soff"] // 2 + 128 * TTOT].rearrange(
        "a (p t) -> (a p) t", p=128)
    dloc_v = bu8[0:1, O["dloc"]:O["dloc"] + 128 * TTOT].rearrange(
        "a (p t) -> (a p) t", p=128)
    dwin_v = bi[0:1, O["dwin"] // 4:O["dwin"] // 4 + 128 * WN].rearrange(
        "a (p w) -> (a p) w", p=128)
    iotaP_v = bf[0:1, O["iotaP"] // 4:O["iotaP"] // 4 + 128].rearrange(
        "a (p c) -> (a p) c", p=128)

    def xt_v(i):
        o = O["xTt"] // 2 + i * F * 128
        return bb[0:1, o:o + F * 128].rearrange("a (f p) -> (a f) p", f=F)

    def w_v(name):
        o = O[name] // 2
        return bb[0:1, o:o + F * ROWC].rearrange("a (f c) -> (a f) c", f=F)

    def dlocf_row(g0, gc):
        o = O["dlocf"] + g0 * 128
        return bu8[0, o:o + gc * 128].partition_broadcast(128)

    out_d = nc.dram_tensor("out", [NPC, F], BF16, kind="ExternalOutput")

    debug = getattr(cfg, "debug", False)
    dbg = "ExternalOutput" if debug else "Internal"
    tab1s_d = nc.dram_tensor("tab1s", [WNP, ROWC], BF16, kind=dbg)
    tab2s_d = nc.dram_tensor("tab2s", [WNP, ROWC], BF16, kind=dbg)
    tab1f_d = nc.dram_tensor(
        "tab1f", [M, WNP, ROWC], BF16, kind="Internal", addr_space="Shared")
    tab2f_d = nc.dram_tensor(
        "tab2f", [M, WNP, ROWC], BF16, kind="Internal", addr_space="Shared")
    if debug:
        dh1_d = nc.dram_tensor(
            "dh1", [WN, F, 128], BF16, kind="ExternalOutput")
        dee_d = nc.dram_tensor(
            "dee", [128, G * H], F32, kind="ExternalOutput")
        derp_d = nc.dram_tensor(
            "derp", [128, G * H], F32, kind="ExternalOutput")
        dps_d = nc.dram_tensor(
            "dps", [128, MC], F32, kind="ExternalOutput")

    with ExitStack() as ctx:
        tc = ctx.enter_context(TileContext(nc))
        const = ctx.enter_context(tc.tile_pool(name="const", bufs=1))
        nxt_p = ctx.enter_context(tc.tile_pool(name="nxt", bufs=4))
        nhb_p = ctx.enter_context(tc.tile_pool(name="nhb", bufs=4))
        rows_p = ctx.enter_context(tc.tile_pool(name="rows", bufs=2))
        ohr_p = ctx.enter_context(tc.tile_pool(name="ohr", bufs=2))
        oh_p = ctx.enter_context(tc.tile_pool(name="oh", bufs=2))
        ohT_p = ctx.enter_context(tc.tile_pool(name="ohT", bufs=2))
        off_p = ctx.enter_context(tc.tile_pool(name="off", bufs=2))
        er_p = ctx.enter_context(tc.tile_pool(name="erp", bufs=4))
        sc_p = ctx.enter_context(tc.tile_pool(name="sc", bufs=2))
        m_p = ctx.enter_context(tc.tile_pool(name="m", bufs=2))
        ep_p = ctx.enter_context(tc.tile_pool(name="ep", bufs=2))
        ps_node = ctx.enter_context(tc.tile_pool(name="psn", bufs=2,
                                                 space="PSUM"))
        ps_agg = ctx.enter_context(tc.tile_pool(name="psa", bufs=3,
                                                space="PSUM"))
        ps_erp = ctx.enter_context(tc.tile_pool(name="pse", bufs=2,
                                                space="PSUM"))
        ps_tr = ctx.enter_context(tc.tile_pool(name="pst", bufs=1,
                                               space="PSUM"))

        # constants
        W1_sb = const.tile([F, ROWC], BF16)
        nc.sync.dma_start(W1_sb[:], w_v("W1"))
        W2_sb = const.tile([F, ROWC], BF16)
        nc.sync.dma_start(W2_sb[:], w_v("W2"))
        b1_sb = const.tile([128, HF], F32)
        nc.sync.dma_start(
            b1_sb[:, :],
            bf[0, O["b1"] // 4:O["b1"] // 4 + HF].partition_broadcast(128))
        b2_sb = const.tile([128, HF], F32)
        nc.sync.dma_start(
            b2_sb[:, :],
            bf[0, O["b2"] // 4:O["b2"] // 4 + HF].partition_broadcast(128))
        iota_sb = const.tile([128, 128], F32)
        nc.sync.dma_start(
            iota_sb[:, :],
            bf[0, O["iotar"] // 4:O["iotar"] // 4 + 128]
            .partition_broadcast(128))
        iotaP_sb = const.tile([128, 1], F32)
        nc.sync.dma_start(iotaP_sb[:], iotaP_v)
        dwin_sb = const.tile([128, WN], I32)
        nc.sync.dma_start(dwin_sb[:], dwin_v)
        ident_sb = const.tile([128, 128], F32)
        make_identity(nc, ident_sb[:])

        def node_phase(tabs_d, W_sb, src_ap, fp8=False):
            for i in range(WN):
                if fp8:
                    xt8 = nxt_p.tile([F, 128], FP8, tag="xt8")
                    nc.sync.dma_start(xt8[:, :], src_ap(i))
                    xt = nxt_p.tile([F, 128], BF16, tag="xt")
                    nc.vector.tensor_copy(xt[:], xt8[:])
                else:
                    xt = nxt_p.tile([F, 128], BF16, tag="xt")
                    nc.sync.dma_start(xt[:, :], src_ap(i))
                ps = ps_node.tile([128, ROWC], F32, tag="nps", name="psnode")
                nc.tensor.matmul(
                    ps[:], lhsT=xt[:, :], rhs=W_sb[:], start=True, stop=True
                )
                hb = nhb_p.tile([128, ROWC], BF16, tag="hb")
                if i % 2 == 0:
                    nc.vector.tensor_copy(hb[:], ps[:])
                else:
                    nc.scalar.activation(hb[:], ps[:], AF.Copy)
                nc.sync.dma_start(tabs_d[i * 128:(i + 1) * 128, :], hb[:])

        def epilogue(layer, w, psw):
            dw = min(128, NPC - w * 128)
            b_sb = b1_sb if layer == 1 else b2_sb
            rec0 = ep_p.tile([128, H], F32, tag="rec0")
            nc.vector.tensor_scalar(
                out=rec0[:], in0=psw[:, HF:HF + H], scalar1=1e-30,
                scalar2=None, op0=ALU.add,
            )
            rec = ep_p.tile([128, H], F32, tag="rec")
            nc.vector.reciprocal(rec[:], rec0[:])
            o2 = ep_p.tile([128, HF], F32, tag="o2")
            nc.vector.tensor_tensor(
                out=o2[:].rearrange("p (h f) -> p h f", h=H),
                in0=psw[:, 0:HF].rearrange("p (h f) -> p h f", h=H),
                in1=rec[:, :].to_broadcast((128, H, F)),
                op=ALU.mult,
            )
            o3 = ep_p.tile([128, HF], F32, tag="o3")
            nc.vector.tensor_tensor(out=o3[:], in0=o2[:], in1=b_sb[:],
                                    op=ALU.add)
            if layer == 1:
                o4 = ep_p.tile([128, HF], F32, tag="o4")
                nc.scalar.activation(o4[:], o3[:], AF.Tanh)
                src_t = o4
            else:
                src_t = o3
            t1 = ep_p.tile([128, 2 * F], F32, tag="t1")
            nc.vector.tensor_tensor(
                out=t1[:], in0=src_t[:, 0:2 * F], in1=src_t[:, 2 * F:4 * F],
                op=ALU.add,
            )
            t2 = ep_p.tile([128, F], F32, tag="t2")
            nc.vector.tensor_tensor(
                out=t2[:], in0=t1[:, 0:F], in1=t1[:, F:2 * F], op=ALU.add
            )
            if layer == 1:
                pst = ps_tr.tile([F, 128], F32, tag="tr", name="pstr")
                nc.tensor.transpose(pst[:], t2[:], ident_sb[:])
                hT = ep_p.tile([F, 128], BF16, tag="hT")
                nc.vector.tensor_scalar_mul(hT[:], pst[:], 1.0 / H)
                if debug:
                    nc.sync.dma_start(dh1_d[w, :, :], hT[:, :])
                ps2 = ps_node.tile([128, ROWC], F32, tag="nps",
                                   name="psnode")
                nc.tensor.matmul(
                    ps2[:], lhsT=hT[:, :], rhs=W2_sb[:],
                    start=True, stop=True)
                hb2 = nhb_p.tile([128, ROWC], BF16, tag="hb")
                if w % 2 == 0:
                    nc.vector.tensor_copy(hb2[:], ps2[:])
                else:
                    nc.scalar.activation(hb2[:], ps2[:], AF.Copy)
                nc.sync.dma_start(
                    tab2s_d[w * 128:(w + 1) * 128, :], hb2[:])
            else:
                om = ep_p.tile([128, F], BF16, tag="om")
                nc.vector.tensor_scalar_mul(om[:], t2[:], 1.0 / H)
                nc.sync.dma_start(out_d[w * 128:w * 128 + dw, :], om[:dw, :])

        def edge_phase(layer, tabf_d):
            tabf = tabf_d[:, :, :].flatten_outer_dims()
            cur_psum = {}
            cur_erw = {}
            g0 = 0
            while g0 < TTOT:
                gc = min(G, TTOT - g0)
                so16 = off_p.tile([128, G], U16, tag="so16")
                nc.sync.dma_start(so16[:, :gc], soff_v[:, g0:g0 + gc])
                so = off_p.tile([128, G], I32, tag="so")
                nc.vector.tensor_copy(so[:, :gc], so16[:, :gc])
                dlt = off_p.tile([128, G], U8, tag="dl")
                nc.sync.dma_start(dlt[:, :gc], dloc_v[:, g0:g0 + gc])
                ohraw = ohr_p.tile([128, G * 128], U8, tag="ohraw")
                nc.sync.dma_start(ohraw[:, :gc * 128], dlocf_row(g0, gc))
                rows = rows_p.tile([128, G, ROWC], BF16, tag="rows")
                for t in range(gc):
                    gt = g0 + t
                    nc.gpsimd.indirect_dma_start(
                        out=rows[:, t, :],
                        out_offset=None,
                        in_=tabf,
                        in_offset=IndirectOffsetOnAxis(
                            ap=so[:, t:t + 1], axis=0
                        ),
                    )
                    if cfg.first_t[gt]:
                        w = cfg.win_of[gt]
                        erw = er_p.tile([128, H], BF16, tag="erw", name="erw")
                        nc.gpsimd.indirect_dma_start(
                            out=erw[:], out_offset=None, in_=tabf,
                            in_offset=IndirectOffsetOnAxis(
                                ap=dwin_sb[:, w:w + 1], axis=0),
                            element_offset=HF + H,
                        )
                        cur_erw[w] = erw
                        cur_psum[w] = ps_agg.tile(
                            [128, MC], F32, tag="agg", name="aggps"
                        )
                # ohT[d, t, e] = (dloc[t,e] == d)
                ohT = ohT_p.tile([128, G, 128], BF16, tag="ohT")
                nc.vector.tensor_scalar(
                    out=ohT[:, :gc, :],
                    in0=ohraw[:, :gc * 128].rearrange(
                        "p (t e) -> p t e", e=128),
                    scalar1=iotaP_sb[:, :], scalar2=None,
                    op0=ALU.is_equal,
                )
                # erp[e, t*4:(t+1)*4] = er[dst_e]
                erp = ps_erp.tile([128, G * H], F32, tag="erp", name="erpps")
                for t in range(gc):
                    w = cfg.win_of[g0 + t]
                    nc.tensor.matmul(
                        erp[:, t * H:(t + 1) * H], lhsT=ohT[:, t, :],
                        rhs=cur_erw[w][:], start=True, stop=True,
                    )
                # chunk-wide scores
                elf = sc_p.tile([128, G * H], F32, tag="elf")
                nc.vector.tensor_copy(
                    elf[:, :gc * H].rearrange("p (t h) -> p t h", h=H),
                    rows[:, :gc, HF:HF + H])
                sc = sc_p.tile([128, G * H], F32, tag="sc")
                nc.vector.tensor_tensor(
                    out=sc[:, :gc * H], in0=elf[:, :gc * H],
                    in1=erp[:, :gc * H], op=ALU.add)
                lr = sc_p.tile([128, G * H], F32, tag="lr")
                nc.vector.scalar_tensor_tensor(
                    out=lr[:, :gc * H], in0=sc[:, :gc * H], scalar=NEG_SLOPE,
                    in1=sc[:, :gc * H], op0=ALU.mult, op1=ALU.max)
                ee = sc_p.tile([128, G * H], F32, tag="ee")
                nc.scalar.activation(ee[:, :gc * H], lr[:, :gc * H], AF.Exp)
                if debug and layer == 1 and g0 == 0:
                    nc.sync.dma_start(dee_d[:, :], ee[:, :])
                    erpc = sc_p.tile([128, G * H], F32, tag="erpc")
                    nc.vector.tensor_copy(erpc[:], erp[:])
                    nc.sync.dma_start(derp_d[:, :], erpc[:, :])
                # m = [h*ee | ee]
                m_t = m_p.tile([128, G, MC], BF16, tag="m")
                nc.vector.tensor_tensor(
                    out=m_t[:, :gc, 0:HF].rearrange(
                        "p t (h f) -> p t h f", h=H),
                    in0=rows[:, :gc, 0:HF].rearrange(
                        "p t (h f) -> p t h f", h=H),
                    in1=ee[:, :gc * H].rearrange("p (t h) -> p t h", h=H)
                        .to_broadcast((128, gc, H, F)),
                    op=ALU.mult,
                )
                nc.vector.tensor_copy(
                    m_t[:, :gc, HF:HF + H],
                    ee[:, :gc * H].rearrange("p (t h) -> p t h", h=H))
                # oh[e, t, d] = (iota[d] == dloc[e, t])
                oh = oh_p.tile([128, G, 128], BF16, tag="oh")
                nc.vector.tensor_tensor(
                    out=oh[:, :gc, :],
                    in0=iota_sb[:, :].unsqueeze(1).to_broadcast(
                        (128, gc, 128)),
                    in1=dlt[:, :gc].to_broadcast((128, gc, 128)),
                    op=ALU.is_equal,
                )
                for t in range(gc):
                    gt = g0 + t
                    w = cfg.win_of[gt]
                    nc.tensor.matmul(
                        cur_psum[w][:],
                        lhsT=oh[:, t, :],
                        rhs=m_t[:, t, :],
                        start=cfg.first_t[gt],
                        stop=cfg.last_t[gt],
                    )
                    if cfg.last_t[gt]:
                        cur_erw.pop(w)
                        psw = cur_psum.pop(w)
                        if debug and layer == 1 and w == 0:
                            psc = ep_p.tile([128, MC], F32, tag="psc")
                            nc.vector.tensor_copy(psc[:], psw[:])
                            nc.sync.dma_start(dps_d[:, :], psc[:, :])
                        epilogue(layer, w, psw[:])
                g0 += gc

        only = getattr(cfg, "only", None)  # None|'n1'|'n1ag'|'n2'
        skip_e = getattr(cfg, "skip_edge", False) or only is not None

        def dummy_out():
            zo = ep_p.tile([128, F], BF16, tag="om")
            nc.vector.memset(zo[:], 0.0)
            nc.sync.dma_start(out_d[0:128, :], zo[:])

        def allgather(tabs_d, tabf_d):
            nc.gpsimd.collective_compute(
                "AllGather",
                ALU.bypass,
                replica_groups=[list(range(M))],
                ins=[tabs_d[:, :]],
                outs=[tabf_d[:, :, :]],
            )

        node_phase(tab1s_d, W1_sb, xt_v)
        if only == "n1":
            dummy_out()
        else:
            allgather(tab1s_d, tab1f_d)
        if not skip_e:
            edge_phase(1, tab1f_d)
        elif only not in ("n1",):
            zz = nhb_p.tile([128, ROWC], BF16, tag="hb")
            nc.vector.memset(zz[:], 0.0)
            nc.sync.dma_start(tab2s_d[0:128, :], zz[:])
        if only not in ("n1", "n1ag"):
            allgather(tab2s_d, tab2f_d)
        if not skip_e:
            edge_phase(2, tab2f_d)
        elif only != "n1":
            dummy_out()

    _cap_dma_waits(nc)
    return nc


def _cap_dma_waits(nc):
    """walrus' pseudo-instruction encodings hold only a couple of sync-wait
    commands (DMA DIRECT2D keeps 1 slot for itself), but Tile can emit more
    (slot WAR + WAW + HWDGE-ring wait). Hoist the excess onto same-engine
    NoOps placed just before the instruction."""
    import bass_rust

    skip = (
        mybir.InstEventSemaphore,
        mybir.InstAllEngineBarrier,
        mybir.InstHalt,
        mybir.InstBranchHint,
    )
    ctr = 0
    for f in nc.m.functions:
        for blk in f.blocks:
            out = []
            changed = False
            for ins in blk.instructions:
                si = ins.sync_info
                if isinstance(ins, skip) or si is None or not si.on_wait:
                    out.append(ins)
                    continue
                cap = 1
                if len(si.on_wait) > cap:
                    waits = list(si.on_wait)
                    extra, keep = waits[:-cap], waits[-cap:]
                    while extra:
                        take, extra = extra[:1], extra[1:]
                        ctr += 1
                        nop = mybir.InstNoOp(
                            name=f"I-waitcap-{ctr}", ins=[], outs=[]
                        )
                        nop.engine = ins.engine
                        nop.sync_info = bass_rust.SyncInfo(
                            on_wait=take, on_update=[]
                        )
                        out.append(nop)
                    ins.sync_info = bass_rust.SyncInfo(
                        on_wait=keep, on_update=list(si.on_update or [])
                    )
                    changed = True
                out.append(ins)
            if changed:
                blk.instructions = out


# ----------------------------------------------------------------------------
# Entry point
# ----------------------------------------------------------------------------
_CACHE = {}


def _run(inputs, trace=False):
    cfg, in_maps = _prepare(**inputs)
    key = (cfg.N, cfg.E, cfg.H, cfg.F, cfg.TTOT, tuple(cfg.TW))
    if key not in _CACHE:
        _CACHE[key] = build_program(cfg)
    nc = _CACHE[key]
    res = run_bass_kernel_spmd(
        nc, in_maps, core_ids=list(range(cfg.M)), trace=trace
    )
    shards = [res.results[c]["out"] for c in range(cfg.M)]
    out = np.concatenate(shards, axis=0).astype(np.float32)
    return out, res


def kernel(**inputs):
    out, _ = _run(inputs, trace=False)
    return out


def hw_time(inputs, iters=20):
    """Estimate per-execution device time: jit once, device-put inputs,
    then (a) sequential blocking calls, (b) pipelined queue of `iters`
    calls with one final block (hides per-call dispatch latency)."""
    import time

    import jax

    from concourse import bass2jax
    from concourse.bass2jax import _bass_exec_p, partition_id_tensor

    cfg, in_maps = _prepare(**inputs)
    key = (cfg.N, cfg.E, cfg.H, cfg.F, cfg.TTOT, tuple(cfg.TW))
    if key not in _CACHE:
        _CACHE[key] = build_program(cfg)
    nc = _CACHE[key]
    bass2jax.install_neuronx_cc_hook()

    partition_name = (
        nc.partition_id_tensor.name if nc.partition_id_tensor else None
    )
    in_names, out_names, out_avals, zero_outs = [], [], [], []
    for alloc in nc.m.functions[0].allocations:
        if not isinstance(alloc, mybir.MemoryLocationSet):
            continue
        name = alloc.memorylocations[0].name
        if alloc.kind == "ExternalInput":
            if name != partition_name:
                in_names.append(name)
        elif alloc.kind == "ExternalOutput":
            shape = tuple(alloc.tensor_shape)
            dtype = mybir.dt.np(alloc.dtype)
            out_avals.append(jax.core.ShapedArray(shape, dtype))
            out_names.append(name)
            zero_outs.append(np.zeros(shape, dtype))
    n_params = len(in_names)
    all_names = list(in_names) + out_names
    if partition_name is not None:
        all_names.append(partition_name)

    def _body(*args):
        operands = list(args)
        if partition_name is not None:
            operands.append(partition_id_tensor())
        outs = _bass_exec_p.bind(
            *operands,
            out_avals=tuple(out_avals),
            in_names=tuple(all_names),
            out_names=tuple(out_names),
            lowering_input_output_aliases=(),
            sim_require_finite=True,
            sim_require_nnan=True,
            nc=nc,
        )
        return tuple(outs)

    from jax.sharding import Mesh, PartitionSpec
    from jax.experimental.shard_map import shard_map

    M = cfg.M
    devices = jax.devices()[:M]
    mesh = Mesh(np.asarray(devices), ("core",))
    in_specs = (PartitionSpec("core"),) * (n_params + len(out_names))
    out_specs = (PartitionSpec("core"),) * len(out_names)
    fn = jax.jit(
        shard_map(
            _body, mesh=mesh, in_specs=in_specs, out_specs=out_specs,
            check_rep=False,
        ),
        keep_unused=True,
    )
    concat_in = [
        np.concatenate([np.asarray(in_maps[c][n]) for c in range(M)], axis=0)
        for n in in_names
    ]
    concat_zero = [
        np.zeros((M * z.shape[0], *z.shape[1:]), z.dtype) for z in zero_outs
    ]
    dev_in = [jax.device_put(a) for a in concat_in]
    dev_zero = [jax.device_put(a) for a in concat_zero]
    r = fn(*dev_in, *dev_zero)
    jax.block_until_ready(r)

    seq = []
    for _ in range(max(5, iters // 4)):
        t0 = time.perf_counter()
        r = fn(*dev_in, *dev_zero)
        jax.block_until_ready(r)
        seq.append(time.perf_counter() - t0)

    t0 = time.perf_counter()
    rs = [fn(*dev_in, *dev_zero) for _ in range(iters)]
    jax.block_until_ready(rs)
    piped = (time.perf_counter() - t0) / iters

    return dict(
        seq_min_s=float(np.min(seq)),
        seq_med_s=float(np.median(seq)),
        piped_avg_s=float(piped),
    )


# revision 28
# speedup vs baseline: 1.2830x; 1.2830x over previous
"""Two-layer GAT (DGL GATConv-style) on 8 Trainium2 NeuronCores via Bass/Tile.

Strategy (v4)
-------------
The per-execution cost through this runner is dominated by host<->device
I/O (inputs are re-shipped every execution; both byte count and tensor
count matter), so the kernel ships ONE ~0.9MiB int32 blob per core
(u16 gather row-ids, u8 dst-local offsets, bf16 features/weights, all
bitcast-sliced on device) and derives everything else on device:

* Node phase is SHARDED: core c projects only its own N/8 nodes
  (one matmul + one PSUM->SBUF copy + one contiguous 67KB write per
  128-node tile) into a local table shard tabs[w*128+p] =
  [h (256) | el (4) | er (4)], all bf16 (528B rows).  An AllGather
  replicates the 3.3MB shard so every core holds the full table; edge
  gathers then index row (c*WN*128 + w*128 + p) for node c*NPC+w*128+p
  (the SAME row mapping serves both layers, so one soff/dwin set).
* Edge phase: edges sorted by dst, windowed by 128 dst nodes, tiled by
  128 edges, chunked by G=32 tiles.  Per chunk: 32 row gathers (indirect
  DMA by src row), a [1, G*128] dloc row partition-broadcast-DMA'd and
  compared against the partition index to build the TRANSPOSED one-hot
  ohT (d-major), per-tile matmuls erp = ohT_t @ er_window broadcast
  er[dst] to edges, then chunk-wide DVE ops: scores, leaky-relu
  (scalar_tensor_tensor), one Exp, m = [h*ee | ee] via stride-0
  broadcast, forward one-hot oh, and per-tile accumulation
  psum_w += oh_t^T @ m_t.  Epilogue per window divides by the summed ee
  (exp(e)/sum exp(e) == softmax exactly), adds bias, tanh+head-mean (L1,
  written transposed for the layer-2 projection) or head-mean (L2).
* Layer 2 projects each core's OWN h1 windows directly (no activation
  collective) and AllGathers the second table shard.
"""

import sys
from contextlib import ExitStack

import numpy as np

sys.path.insert(0, "/opt/trn_rl_repo")

import concourse.bass as bass  # noqa: E402
import concourse.mybir as mybir  # noqa: E402
from concourse.bass import IndirectOffsetOnAxis  # noqa: E402
from concourse.bass_utils import run_bass_kernel_spmd  # noqa: E402
from concourse.masks import make_identity  # noqa: E402
from concourse.tile import TileContext  # noqa: E402

BF16 = mybir.dt.bfloat16
F32 = mybir.dt.float32
I32 = mybir.dt.int32
U16 = mybir.dt.uint16
U8 = mybir.dt.uint8
FP8 = mybir.dt.float8e4
NP_BF16 = mybir.dt.np(BF16)
NP_FP8 = mybir.dt.np(FP8)

AF = mybir.ActivationFunctionType
ALU = mybir.AluOpType

M_CORES = 8
NEG_SLOPE = 0.2
G_TILES = 32  # tiles (of 128 edges) per chunk


# ----------------------------------------------------------------------------
# Host-side preprocessing
# ----------------------------------------------------------------------------
class Cfg:
    pass


def _ceil_div(a, b):
    return -(-a // b)


def _prepare(x, src, dst, W1, al1, ar1, b1, W2, al2, ar2, b2, m_cores=M_CORES):
    cfg = Cfg()
    N, F = x.shape
    E = src.shape[0]
    H = al1.shape[0]
    assert N % m_cores == 0
    npc = N // m_cores
    wn = _ceil_div(npc, 128)
    wnp = wn * 128
    HF = H * F

    cfg.N, cfg.F, cfg.E, cfg.H, cfg.M = N, F, E, H, m_cores
    cfg.NPC, cfg.WN, cfg.WNP, cfg.HF = npc, wn, wnp, HF
    cfg.ROWC = HF + 2 * H  # bf16 cols: h | el | er
    cfg.MC = HF + H  # matmul rhs cols: scaled h | ee

    # unified table row id for node n (both layers)
    def rid(n):
        c = n // npc
        l = n % npc
        return c * wnp + (l // 128) * 128 + l % 128

    # ---- edge partition: sort by dst, split by dst range, window by 128 ----
    order = np.argsort(dst, kind="stable")
    ss = src[order].astype(np.int64)
    ds = dst[order].astype(np.int64)
    core = ds // npc
    dl = ds % npc
    win = dl // 128
    dloc = (dl - win * 128).astype(np.float32)

    grp = core * wn + win  # non-decreasing
    counts = np.bincount(grp, minlength=m_cores * wn).reshape(m_cores, wn)
    tw = np.maximum(1, _ceil_div(counts.max(axis=0), 128))  # tiles per window
    ttot = int(tw.sum())
    base = np.zeros(wn + 1, np.int64)
    base[1:] = np.cumsum(tw * 128)
    starts = np.searchsorted(grp, np.arange(m_cores * wn))
    ends = np.searchsorted(grp, np.arange(m_cores * wn) + 1)

    soff = np.zeros((m_cores, 128, ttot), np.uint16)
    dlocs = np.zeros((m_cores, 128, ttot), np.uint8)
    dlocf = np.zeros((m_cores, 1, ttot * 128), np.uint8)
    for c in range(m_cores):
        s_src = np.zeros(ttot * 128, np.int64)
        s_dlc = np.full(ttot * 128, 255, np.int64)
        for w in range(wn):
            s0, e0 = starts[c * wn + w], ends[c * wn + w]
            n = e0 - s0
            b0 = base[w]
            s_src[b0:b0 + n] = ss[s0:e0]
            s_dlc[b0:b0 + n] = dloc[s0:e0].astype(np.int64)
        soff[c] = rid(s_src).reshape(ttot, 128).T.astype(np.uint16)
        dlocs[c] = s_dlc.reshape(ttot, 128).T.astype(np.uint8)
        dlocf[c, 0] = s_dlc.astype(np.uint8)

    p_ar = np.arange(128)
    dwin = np.zeros((m_cores, 128, wn), np.int32)
    for c in range(m_cores):
        for w in range(wn):
            dw = min(128, npc - w * 128)
            dwin[c, :, w] = c * wnp + w * 128 + np.minimum(p_ar, dw - 1)

    cfg.TW = [int(t) for t in tw]
    cfg.TTOT = ttot
    win_of, first_t, last_t = [], [], []
    for w in range(wn):
        for i in range(cfg.TW[w]):
            win_of.append(w)
            first_t.append(i == 0)
            last_t.append(i == cfg.TW[w] - 1)
    cfg.win_of, cfg.first_t, cfg.last_t = win_of, first_t, last_t

    # ---- folded weights: [W | W.al | W.ar] -> [F, ROWC] bf16 ----
    def aug(Wm, al, ar):
        W64 = Wm.astype(np.float64).reshape(F, H, F)
        wal = np.einsum("khf,hf->kh", W64, al.astype(np.float64))
        war = np.einsum("khf,hf->kh", W64, ar.astype(np.float64))
        return np.concatenate(
            [Wm.astype(np.float64), wal, war], axis=1
        ).astype(NP_BF16)

    W1a = aug(W1, al1, ar1)
    W2a = aug(W2, al2, ar2)

    b1row = np.asarray(b1, np.float32).reshape(HF)
    b2row = np.asarray(b2, np.float32).reshape(HF)
    iotar = np.arange(128, dtype=np.float32)
    iotaP = np.arange(128, dtype=np.float32)

    # ---- pack everything into ONE int32 blob per core (per-exec transfer
    # cost scales with tensor COUNT as well as bytes) ----
    off = {}
    cur = 0

    def _add(name, nbytes):
        nonlocal cur
        cur = (cur + 511) // 512 * 512
        off[name] = cur
        cur += nbytes

    _add("soff", 128 * ttot * 2)
    _add("dloc", 128 * ttot * 1)
    _add("dlocf", ttot * 128 * 1)
    _add("xTt", wnp * F * 2)
    _add("W1", F * cfg.ROWC * 2)
    _add("W2", F * cfg.ROWC * 2)
    _add("b1", HF * 4)
    _add("b2", HF * 4)
    _add("iotar", 128 * 4)
    _add("iotaP", 128 * 4)
    _add("dwin", 128 * wn * 4)
    totb = (cur + 511) // 512 * 512
    cfg.off, cfg.TOTB = off, totb

    in_maps = []
    for c in range(m_cores):
        xs = np.zeros((wnp, F), np.float32)
        xs[:npc] = np.asarray(x[c * npc:(c + 1) * npc], np.float32)
        xTt = np.ascontiguousarray(
            xs.reshape(wn, 128, F).transpose(0, 2, 1)).astype(NP_BF16)
        buf = np.zeros(totb, np.uint8)

        def _put(name, arr):
            b = arr.tobytes()
            buf[off[name]:off[name] + len(b)] = np.frombuffer(b, np.uint8)

        _put("soff", np.ascontiguousarray(soff[c]))
        _put("dloc", np.ascontiguousarray(dlocs[c]))
        _put("dlocf", np.ascontiguousarray(dlocf[c]))
        _put("xTt", xTt)
        _put("W1", W1a)
        _put("W2", W2a)
        _put("b1", b1row)
        _put("b2", b2row)
        _put("iotar", iotar)
        _put("iotaP", iotaP)
        _put("dwin", np.ascontiguousarray(dwin[c]))
        in_maps.append(dict(blob=buf.view(np.int32).reshape(1, -1)))
    return cfg, in_maps


# ----------------------------------------------------------------------------
# Bass program
# ----------------------------------------------------------------------------
def build_program(cfg):
    N, F, H, M = cfg.N, cfg.F, cfg.H, cfg.M
    HF, NPC, WN, WNP = cfg.HF, cfg.NPC, cfg.WN, cfg.WNP
    ROWC, MC = cfg.ROWC, cfg.MC
    G = G_TILES
    TTOT = cfg.TTOT

    nc = bass.Bass(num_devices=M, enable_partition_id=False)

    O = cfg.off
    blob_d = nc.dram_tensor(
        "blob", [1, cfg.TOTB // 4], I32, kind="ExternalInput")
    bb = blob_d.bitcast(BF16)
    bi = blob_d
    bf = blob_d.bitcast(F32)
    bu16 = blob_d.bitcast(U16)
    bu8 = blob_d.bitcast(U8)
    bf8 = blob_d.bitcast(FP8)
    soff_v = bu16[0:1, O["soff"] // 2:O["soff"] // 2 + 128 * TTOT].rearrange(
        "a (p t) -> (a p) t", p=128)
    dloc_v = bu8[0:1, O["dloc"]:O["dloc"] + 128 * TTOT].rearrange(
        "a (p t) -> (a p) t", p=128)
    dwin_v = bi[0:1, O["dwin"] // 4:O["dwin"] // 4 + 128 * WN].rearrange(
        "a (p w) -> (a p) w", p=128)
    iotaP_v = bf[0:1, O["iotaP"] // 4:O["iotaP"] // 4 + 128].rearrange(
        "a (p c) -> (a p) c", p=128)

    def xt_v(i):
        o = O["xTt"] // 2 + i * F * 128
        return bb[0:1, o:o + F * 128].rearrange("a (f p) -> (a f) p", f=F)

    def w_v(name):
        o = O[name] // 2
        return bb[0:1, o:o + F * ROWC].rearrange("a (f c) -> (a f) c", f=F)

    def dlocf_row(g0, gc):
        o = O["dlocf"] + g0 * 128
        return bu8[0, o:o + gc * 128].partition_broadcast(128)

    out_d = nc.dram_tensor("out", [NPC, F], BF16, kind="ExternalOutput")

    debug = getattr(cfg, "debug", False)
    dbg = "ExternalOutput" if debug else "Internal"
    tab1s_d = nc.dram_tensor("tab1s", [WNP, ROWC], BF16, kind=dbg)
    tab2s_d = nc.dram_tensor("tab2s", [WNP, ROWC], BF16, kind=dbg)
    tab1f_d = nc.dram_tensor(
        "tab1f", [M, WNP, ROWC], BF16, kind="Internal", addr_space="Shared")
    tab2f_d = nc.dram_tensor(
        "tab2f", [M, WNP, ROWC], BF16, kind="Internal", addr_space="Shared")
    if debug:
        dh1_d = nc.dram_tensor(
            "dh1", [WN, F, 128], BF16, kind="ExternalOutput")
        dee_d = nc.dram_tensor(
            "dee", [128, G * H], F32, kind="ExternalOutput")
        derp_d = nc.dram_tensor(
            "derp", [128, G * H], F32, kind="ExternalOutput")
        dps_d = nc.dram_tensor(
            "dps", [128, MC], F32, kind="ExternalOutput")

    with ExitStack() as ctx:
        tc = ctx.enter_context(TileContext(nc))
        const = ctx.enter_context(tc.tile_pool(name="const", bufs=1))
        nxt_p = ctx.enter_context(tc.tile_pool(name="nxt", bufs=4))
        nhb_p = ctx.enter_context(tc.tile_pool(name="nhb", bufs=4))
        rows_p = ctx.enter_context(tc.tile_pool(name="rows", bufs=2))
        ohr_p = ctx.enter_context(tc.tile_pool(name="ohr", bufs=2))
        oh_p = ctx.enter_context(tc.tile_pool(name="oh", bufs=2))
        ohT_p = ctx.enter_context(tc.tile_pool(name="ohT", bufs=2))
        off_p = ctx.enter_context(tc.tile_pool(name="off", bufs=2))
        er_p = ctx.enter_context(tc.tile_pool(name="erp", bufs=4))
        sc_p = ctx.enter_context(tc.tile_pool(name="sc", bufs=2))
        m_p = ctx.enter_context(tc.tile_pool(name="m", bufs=2))
        ep_p = ctx.enter_context(tc.tile_pool(name="ep", bufs=2))
        ps_node = ctx.enter_context(tc.tile_pool(name="psn", bufs=2,
                                                 space="PSUM"))
        ps_agg = ctx.enter_context(tc.tile_pool(name="psa", bufs=3,
                                                space="PSUM"))
        ps_erp = ctx.enter_context(tc.tile_pool(name="pse", bufs=2,
                                                space="PSUM"))
        ps_tr = ctx.enter_context(tc.tile_pool(name="pst", bufs=1,
                                               space="PSUM"))

        # constants
        W1_sb = const.tile([F, ROWC], BF16)
        nc.sync.dma_start(W1_sb[:], w_v("W1"))
        W2_sb = const.tile([F, ROWC], BF16)
        nc.sync.dma_start(W2_sb[:], w_v("W2"))
        b1_sb = const.tile([128, HF], F32)
        nc.sync.dma_start(
            b1_sb[:, :],
            bf[0, O["b1"] // 4:O["b1"] // 4 + HF].partition_broadcast(128))
        b2_sb = const.tile([128, HF], F32)
        nc.sync.dma_start(
            b2_sb[:, :],
            bf[0, O["b2"] // 4:O["b2"] // 4 + HF].partition_broadcast(128))
        iota_sb = const.tile([128, 128], F32)
        nc.sync.dma_start(
            iota_sb[:, :],
            bf[0, O["iotar"] // 4:O["iotar"] // 4 + 128]
            .partition_broadcast(128))
        iotaP_sb = const.tile([128, 1], F32)
        nc.sync.dma_start(iotaP_sb[:], iotaP_v)
        dwin_sb = const.tile([128, WN], I32)
        nc.sync.dma_start(dwin_sb[:], dwin_v)
        ident_sb = const.tile([128, 128], F32)
        make_identity(nc, ident_sb[:])

        def node_phase(tabs_d, W_sb, src_ap, fp8=False):
            for i in range(WN):
                if fp8:
                    xt8 = nxt_p.tile([F, 128], FP8, tag="xt8")
                    nc.sync.dma_start(xt8[:, :], src_ap(i))
                    xt = nxt_p.tile([F, 128], BF16, tag="xt")
                    nc.vector.tensor_copy(xt[:], xt8[:])
                else:
                    xt = nxt_p.tile([F, 128], BF16, tag="xt")
                    nc.sync.dma_start(xt[:, :], src_ap(i))
                ps = ps_node.tile([128, ROWC], F32, tag="nps", name="psnode")
                nc.tensor.matmul(
                    ps[:], lhsT=xt[:, :], rhs=W_sb[:], start=True, stop=True
                )
                hb = nhb_p.tile([128, ROWC], BF16, tag="hb")
                if i % 2 == 0:
                    nc.vector.tensor_copy(hb[:], ps[:])
                else:
                    nc.scalar.activation(hb[:], ps[:], AF.Copy)
                nc.sync.dma_start(tabs_d[i * 128:(i + 1) * 128, :], hb[:])

        def epilogue(layer, w, psw):
            dw = min(128, NPC - w * 128)
            b_sb = b1_sb if layer == 1 else b2_sb
            rec0 = ep_p.tile([128, H], F32, tag="rec0")
            nc.vector.tensor_scalar(
                out=rec0[:], in0=psw[:, HF:HF + H], scalar1=1e-30,
                scalar2=None, op0=ALU.add,
            )
            rec = ep_p.tile([128, H], F32, tag="rec")
            nc.vector.reciprocal(rec[:], rec0[:])
            o2 = ep_p.tile([128, HF], F32, tag="o2")
            nc.vector.tensor_tensor(
                out=o2[:].rearrange("p (h f) -> p h f", h=H),
                in0=psw[:, 0:HF].rearrange("p (h f) -> p h f", h=H),
                in1=rec[:, :].to_broadcast((128, H, F)),
                op=ALU.mult,
            )
            o3 = ep_p.tile([128, HF], F32, tag="o3")
            nc.vector.tensor_tensor(out=o3[:], in0=o2[:], in1=b_sb[:],
                                    op=ALU.add)
            if layer == 1:
                o4 = ep_p.tile([128, HF], F32, tag="o4")
                nc.scalar.activation(o4[:], o3[:], AF.Tanh)
                src_t = o4
            else:
                src_t = o3
            t1 = ep_p.tile([128, 2 * F], F32, tag="t1")
            nc.vector.tensor_tensor(
                out=t1[:], in0=src_t[:, 0:2 * F], in1=src_t[:, 2 * F:4 * F],
                op=ALU.add,
            )
            t2 = ep_p.tile([128, F], F32, tag="t2")
            nc.vector.tensor_tensor(
                out=t2[:], in0=t1[:, 0:F], in1=t1[:, F:2 * F], op=ALU.add
            )
            if layer == 1:
                pst = ps_tr.tile([F, 128], F32, tag="tr", name="pstr")
                nc.tensor.transpose(pst[:], t2[:], ident_sb[:])
                hT = ep_p.tile([F, 128], BF16, tag="hT")
                nc.vector.tensor_scalar_mul(hT[:], pst[:], 1.0 / H)
                if debug:
                    nc.sync.dma_start(dh1_d[w, :, :], hT[:, :])
                ps2 = ps_node.tile([128, ROWC], F32, tag="nps",
                                   name="psnode")
                nc.tensor.matmul(
                    ps2[:], lhsT=hT[:, :], rhs=W2_sb[:],
                    start=True, stop=True)
                hb2 = nhb_p.tile([128, ROWC], BF16, tag="hb")
                if w % 2 == 0:
                    nc.vector.tensor_copy(hb2[:], ps2[:])
                else:
                    nc.scalar.activation(hb2[:], ps2[:], AF.Copy)
                nc.sync.dma_start(
                    tab2s_d[w * 128:(w + 1) * 128, :], hb2[:])
            else:
                om = ep_p.tile([128, F], BF16, tag="om")
                nc.vector.tensor_scalar_mul(om[:], t2[:], 1.0 / H)
                nc.sync.dma_start(out_d[w * 128:w * 128 + dw, :], om[:dw, :])

        def edge_phase(layer, tabf_d):
            tabf = tabf_d[:, :, :].flatten_outer_dims()
            cur_psum = {}
            cur_erw = {}
            g0 = 0
            while g0 < TTOT:
                gc = min(G, TTOT - g0)
                so16 = off_p.tile([128, G], U16, tag="so16")
                nc.sync.dma_start(so16[:, :gc], soff_v[:, g0:g0 + gc])
                so = off_p.tile([128, G], I32, tag="so")
                nc.vector.tensor_copy(so[:, :gc], so16[:, :gc])
                dlt = off_p.tile([128, G], U8, tag="dl")
                nc.sync.dma_start(dlt[:, :gc], dloc_v[:, g0:g0 + gc])
                ohraw = ohr_p.tile([128, G * 128], U8, tag="ohraw")
                nc.sync.dma_start(ohraw[:, :gc * 128], dlocf_row(g0, gc))
                rows = rows_p.tile([128, G, ROWC], BF16, tag="rows")
                for t in range(gc):
                    gt = g0 + t
                    nc.gpsimd.indirect_dma_start(
                        out=rows[:, t, :],
                        out_offset=None,
                        in_=tabf,
                        in_offset=IndirectOffsetOnAxis(
                            ap=so[:, t:t + 1], axis=0
                        ),
                    )
                    if cfg.first_t[gt]:
                        w = cfg.win_of[gt]
                        erw = er_p.tile([128, H], BF16, tag="erw", name="erw")
                        nc.gpsimd.indirect_dma_start(
                            out=erw[:], out_offset=None, in_=tabf,
                            in_offset=IndirectOffsetOnAxis(
                                ap=dwin_sb[:, w:w + 1], axis=0),
                            element_offset=HF + H,
                        )
                        cur_erw[w] = erw
                        cur_psum[w] = ps_agg.tile(
                            [128, MC], F32, tag="agg", name="aggps"
                        )
                # ohT[d, t, e] = (dloc[t,e] == d)
                ohT = ohT_p.tile([128, G, 128], BF16, tag="ohT")
                nc.vector.tensor_scalar(
                    out=ohT[:, :gc, :],
                    in0=ohraw[:, :gc * 128].rearrange(
                        "p (t e) -> p t e", e=128),
                    scalar1=iotaP_sb[:, :], scalar2=None,
                    op0=ALU.is_equal,
                )
                # erp[e, t*4:(t+1)*4] = er[dst_e]
                erp = ps_erp.tile([128, G * H], F32, tag="erp", name="erpps")
                for t in range(gc):
                    w = cfg.win_of[g0 + t]
                    nc.tensor.matmul(
                        erp[:, t * H:(t + 1) * H], lhsT=ohT[:, t, :],
                        rhs=cur_erw[w][:], start=True, stop=True,
                    )
                # chunk-wide scores
                elf = sc_p.tile([128, G * H], F32, tag="elf")
                nc.vector.tensor_copy(
                    elf[:, :gc * H].rearrange("p (t h) -> p t h", h=H),
                    rows[:, :gc, HF:HF + H])
                sc = sc_p.tile([128, G * H], F32, tag="sc")
                nc.vector.tensor_tensor(
                    out=sc[:, :gc * H], in0=elf[:, :gc * H],
                    in1=erp[:, :gc * H], op=ALU.add)
                lr = sc_p.tile([128, G * H], F32, tag="lr")
                nc.vector.scalar_tensor_tensor(
                    out=lr[:, :gc * H], in0=sc[:, :gc * H], scalar=NEG_SLOPE,
                    in1=sc[:, :gc * H], op0=ALU.mult, op1=ALU.max)
                ee = sc_p.tile([128, G * H], F32, tag="ee")
                nc.scalar.activation(ee[:, :gc * H], lr[:, :gc * H], AF.Exp)
                if debug and layer == 1 and g0 == 0:
                    nc.sync.dma_start(dee_d[:, :], ee[:, :])
                    erpc = sc_p.tile([128, G * H], F32, tag="erpc")
                    nc.vector.tensor_copy(erpc[:], erp[:])
                    nc.sync.dma_start(derp_d[:, :], erpc[:, :])
                # m = [h*ee | ee]
                m_t = m_p.tile([128, G, MC], BF16, tag="m")
                nc.vector.tensor_tensor(
                    out=m_t[:, :gc, 0:HF].rearrange(
                        "p t (h f) -> p t h f", h=H),
                    in0=rows[:, :gc, 0:HF].rearrange(
                        "p t (h f) -> p t h f", h=H),
                    in1=ee[:, :gc * H].rearrange("p (t h) -> p t h", h=H)
                        .to_broadcast((128, gc, H, F)),
                    op=ALU.mult,
                )
                nc.vector.tensor_copy(
                    m_t[:, :gc, HF:HF + H],
                    ee[:, :gc * H].rearrange("p (t h) -> p t h", h=H))
                # oh[e, t, d] = (iota[d] == dloc[e, t])
                oh = oh_p.tile([128, G, 128], BF16, tag="oh")
                nc.vector.tensor_tensor(
                    out=oh[:, :gc, :],
                    in0=iota_sb[:, :].unsqueeze(1).to_broadcast(
                        (128, gc, 128)),
                    in1=dlt[:, :gc].to_broadcast((128, gc, 128)),
                    op=ALU.is_equal,
                )
                for t in range(gc):
                    gt = g0 + t
                    w = cfg.win_of[gt]
                    nc.tensor.matmul(
                        cur_psum[w][:],
                        lhsT=oh[:, t, :],
                        rhs=m_t[:, t, :],
                        start=cfg.first_t[gt],
                        stop=cfg.last_t[gt],
                    )
                    if cfg.last_t[gt]:
                        cur_erw.pop(w)
                        psw = cur_psum.pop(w)
                        if debug and layer == 1 and w == 0:
                            psc = ep_p.tile([128, MC], F32, tag="psc")
                            nc.vector.tensor_copy(psc[:], psw[:])
                            nc.sync.dma_start(dps_d[:, :], psc[:, :])
                        epilogue(layer, w, psw[:])
                g0 += gc

        only = getattr(cfg, "only", None)  # None|'n1'|'n1ag'|'n2'
        skip_e = getattr(cfg, "skip_edge", False) or only is not None

        def dummy_out():
            zo = ep_p.tile([128, F], BF16, tag="om")
            nc.vector.memset(zo[:], 0.0)
            nc.sync.dma_start(out_d[0:128, :], zo[:])

        def allgather(tabs_d, tabf_d):
            nc.gpsimd.collective_compute(
                "AllGather",
                ALU.bypass,
                replica_groups=[list(range(M))],
                ins=[tabs_d[:, :]],
                outs=[tabf_d[:, :, :]],
            )

        node_phase(tab1s_d, W1_sb, xt_v)
        if only == "n1":
            dummy_out()
        else:
            allgather(tab1s_d, tab1f_d)
        if not skip_e:
            edge_phase(1, tab1f_d)
        elif only not in ("n1",):
            zz = nhb_p.tile([128, ROWC], BF16, tag="hb")
            nc.vector.memset(zz[:], 0.0)
            nc.sync.dma_start(tab2s_d[0:128, :], zz[:])
        if only not in ("n1", "n1ag"):
            allgather(tab2s_d, tab2f_d)
        if not skip_e:
            edge_phase(2, tab2f_d)
        elif only != "n1":
            dummy_out()

    _cap_dma_waits(nc)
    return nc


def _cap_dma_waits(nc):
    """walrus' pseudo-instruction encodings hold only a couple of sync-wait
    commands (DMA DIRECT2D keeps 1 slot for itself), but Tile can emit more
    (slot WAR + WAW + HWDGE-ring wait). Hoist the excess onto same-engine
    NoOps placed just before the instruction."""
    import bass_rust

    skip = (
        mybir.InstEventSemaphore,
        mybir.InstAllEngineBarrier,
        mybir.InstHalt,
        mybir.InstBranchHint,
    )
    ctr = 0
    for f in nc.m.functions:
        for blk in f.blocks:
            out = []
            changed = False
            for ins in blk.instructions:
                si = ins.sync_info
                if isinstance(ins, skip) or si is None or not si.on_wait:
                    out.append(ins)
                    continue
                cap = 1
                if len(si.on_wait) > cap:
                    waits = list(si.on_wait)
                    extra, keep = waits[:-cap], waits[-cap:]
                    while extra:
                        take, extra = extra[:1], extra[1:]
                        ctr += 1
                        nop = mybir.InstNoOp(
                            name=f"I-waitcap-{ctr}", ins=[], outs=[]
                        )
                        nop.engine = ins.engine
                        nop.sync_info = bass_rust.SyncInfo(
                            on_wait=take, on_update=[]
                        )
                        out.append(nop)
                    ins.sync_info = bass_rust.SyncInfo(
                        on_wait=keep, on_update=list(si.on_update or [])
                    )
                    changed = True
                out.append(ins)
            if changed:
                blk.instructions = out


# ----------------------------------------------------------------------------
# Entry point
# ----------------------------------------------------------------------------
_CACHE = {}


def _run(inputs, trace=False):
    cfg, in_maps = _prepare(**inputs)
    key = (cfg.N, cfg.E, cfg.H, cfg.F, cfg.TTOT, tuple(cfg.TW))
    if key not in _CACHE:
        _CACHE[key] = build_program(cfg)
    nc = _CACHE[key]
    res = run_bass_kernel_spmd(
        nc, in_maps, core_ids=list(range(cfg.M)), trace=trace
    )
    shards = [res.results[c]["out"] for c in range(cfg.M)]
    out = np.concatenate(shards, axis=0).astype(np.float32)
    return out, res


def kernel(**inputs):
    out, _ = _run(inputs, trace=False)
    return out


def hw_time(inputs, iters=20):
    """Estimate per-execution device time: jit once, device-put inputs,
    then (a) sequential blocking calls, (b) pipelined queue of `iters`
    calls with one final block (hides per-call dispatch latency)."""
    import time

    import jax

    from concourse import bass2jax
    from concourse.bass2jax import _bass_exec_p, partition_id_tensor

    cfg, in_maps = _prepare(**inputs)
    key = (cfg.N, cfg.E, cfg.H, cfg.F, cfg.TTOT, tuple(cfg.TW))
    if key not in _CACHE:
        _CACHE[key] = build_program(cfg)
    nc = _CACHE[key]
    bass2jax.install_neuronx_cc_hook()

    partition_name = (
        nc.partition_id_tensor.name if nc.partition_id_tensor else None
    )
    in_names, out_names, out_avals, zero_outs = [], [], [], []
    for alloc in nc.m.functions[0].allocations:
        if not isinstance(alloc, mybir.MemoryLocationSet):
            continue
        name = alloc.memorylocations[0].name
        if alloc.kind == "ExternalInput":
            if name != partition_name:
                in_names.append(name)
        elif alloc.kind == "ExternalOutput":
            shape = tuple(alloc.tensor_shape)
            dtype = mybir.dt.np(alloc.dtype)
            out_avals.append(jax.core.ShapedArray(shape, dtype))
            out_names.append(name)
            zero_outs.append(np.zeros(shape, dtype))
    n_params = len(in_names)
    all_names = list(in_names) + out_names
    if partition_name is not None:
        all_names.append(partition_name)

    def _body(*args):
        operands = list(args)
        if partition_name is not None:
            operands.append(partition_id_tensor())
        outs = _bass_exec_p.bind(
            *operands,
            out_avals=tuple(out_avals),
            in_names=tuple(all_names),
            out_names=tuple(out_names),
            lowering_input_output_aliases=(),
            sim_require_finite=True,
            sim_require_nnan=True,
            nc=nc,
        )
        return tuple(outs)

    from jax.sharding import Mesh, PartitionSpec
    from jax.experimental.shard_map import shard_map

    M = cfg.M
    devices = jax.devices()[:M]
    mesh = Mesh(np.asarray(devices), ("core",))
    in_specs = (PartitionSpec("core"),) * (n_params + len(out_names))
    out_specs = (PartitionSpec("core"),) * len(out_names)
    fn = jax.jit(
        shard_map(
            _body, mesh=mesh, in_specs=in_specs, out_specs=out_specs,
            check_rep=False,
        ),
        keep_unused=True,
    )
    concat_in = [
        np.concatenate([np.asarray(in_maps[c][n]) for c in range(M)], axis=0)
        for n in in_names
    ]
    concat_zero = [
        np.zeros((M * z.shape[0], *z.shape[1:]), z.dtype) for z in zero_outs
    ]
    dev_in = [jax.device_put(a) for a in concat_in]
    dev_zero = [jax.device_put(a) for a in concat_zero]
    r = fn(*dev_in, *dev_zero)
    jax.block_until_ready(r)

    seq = []
    for _ in range(max(5, iters // 4)):
        t0 = time.perf_counter()
        r = fn(*dev_in, *dev_zero)
        jax.block_until_ready(r)
        seq.append(time.perf_counter() - t0)

    t0 = time.perf_counter()
    rs = [fn(*dev_in, *dev_zero) for _ in range(iters)]
    jax.block_until_ready(rs)
    piped = (time.perf_counter() - t0) / iters

    return dict(
        seq_min_s=float(np.min(seq)),
        seq_med_s=float(np.median(seq)),
        piped_avg_s=float(piped),
    )
